# revision 4
# baseline (speedup 1.0000x reference)
"""Trainium2 Bass kernel for IR-Net style binarized conv block.

Computation (matches the reference nn.Module):
  1. Per-out-channel weight standardization -> sign -> {-1,+1}, power-of-2
     per-channel scale sw (host-side numpy; weights are tiny).
  2. ba = sign(x) (device, ScalarE Sign activation, exact in fp8).
  3. y = conv2d(ba, sign_w) * sw  -- 3x3, pad 1, stride 1. Done as 9 shifted
     matmuls over a zero-padded SBUF activation image, channels on the
     partition axis, accumulating in PSUM. fp8 DoubleRow contracts both
     128-channel input groups at once. Exact: products are +-1 summed in
     fp32 PSUM.
  4. Training-mode BatchNorm over the FULL batch: per-channel sum / sumsq
     accumulate on-device, AllGather'd across the 8 cores (1KB), folded with
     sw, gamma, beta into per-channel affine a*z - bneg.
  5. Hardtanh clip via tensor_scalar(min,max).

Sharding: pure data parallel, batch 32 -> 4 images per core x 8 cores.

v2 over baseline:
  - WPAD=58 (=w+2, minimal padding; HP=64 keeps planes 16B-aligned) cuts the
    garbage columns in each matmul from 32/480 to 16/464.
  - Startup: image-0 quarter-0 DMAs + binarize are emitted first so the
    first matmul can start ~4us in instead of ~17us.
  - Stats bounce DMAs ride the Pool queue (Act was blocked on an
    ACT_TABLE_LOAD right when the tail collective had to launch).
  - Normalize (affine+clip) is split across DVE / Act / Pool per image so
    the tail isn't serialized on DVE.
  - Coefficient chain is fewer ops, mostly on DVE.
"""

import numpy as np
import ml_dtypes

import concourse.bacc as bacc
import concourse.bass as bass
import concourse.tile as tile
from concourse import mybir
from concourse.bass_utils import run_bass_kernel_spmd

F32 = mybir.dt.float32
FP8 = mybir.dt.float8e4

P = 128          # SBUF partitions
CG = 2           # channel groups: 256 channels = 2 x 128
C = 256
BN_EPS = 1e-5
N_CORES = 8
RT = 8           # output rows per PSUM tile


def build_kernel(b_per_core=4, h=56, w=56, n_cores=N_CORES):
    """Build the per-core Bass program. Returns the compiled Bacc instance."""
    WPAD = w + 2                        # 58: exactly left pad + data + right pad
    # fp8 DoubleRow moving AP is a flat contiguous block that can overrun the
    # last padded row by up to kh*WPAD+kw; give each image spare zero rows and
    # keep the per-image plane 16B-aligned (HP*WPAD % 16 == 0).
    HP = 64
    assert (HP * WPAD) % 16 == 0 and HP >= h + 4
    assert h % RT == 0
    tiles_per_img = h // RT
    NT = b_per_core * tiles_per_img     # PSUM tiles per output-channel group
    FREE_PS = RT * WPAD                 # moving free dim per matmul (464)
    FREE = RT * w                       # useful elems per tile (448)
    assert FREE_PS <= 512
    nhw_total = n_cores * b_per_core * h * w

    nc = bacc.Bacc(
        "TRN2", target_bir_lowering=False, debug=False, num_devices=n_cores
    )
    x_d = nc.dram_tensor("x", [b_per_core, C, h, w], F32, kind="ExternalInput").ap()
    w_d = nc.dram_tensor("wsgn", [P, CG, 9, C], FP8, kind="ExternalInput").ap()
    coef_d = nc.dram_tensor("coef", [P, CG, 3], F32, kind="ExternalInput").ap()
    out_d = nc.dram_tensor(
        "out", [b_per_core, C, h, w], F32, kind="ExternalOutput"
    ).ap()

    mult = mybir.AluOpType.mult
    add = mybir.AluOpType.add
    subtract = mybir.AluOpType.subtract
    amin = mybir.AluOpType.min
    amax = mybir.AluOpType.max
    AF = mybir.ActivationFunctionType
    NQ = 4
    hh = h // NQ                        # binarize chunk rows (14)

    with tile.TileContext(nc) as tc:
        with (
            tc.tile_pool(name="singles", bufs=1) as singles,
            tc.tile_pool(name="xs", bufs=4) as xs_pool,
            tc.tile_pool(name="psum", bufs=8, space="PSUM") as psum_pool,
            tc.tile_pool(name="sq", bufs=2) as sq_pool,
            tc.tile_pool(name="stage", bufs=2) as stage_pool,
            tc.tile_pool(name="small", bufs=1) as small,
            tc.tile_pool(name="dram", bufs=1, space="DRAM") as dram,
        ):
            # ---- padded, binarized activations (resident) ----
            acts = singles.tile([P, CG, b_per_core, HP, WPAD], FP8)

            def emit_zero_margins(n, eng):
                eng.memset(acts[:, :, n, 0, :], 0.0)                  # top pad
                eng.memset(acts[:, :, n, h + 1 : h + 5, :], 0.0)      # bottom pad + spill rows
                eng.memset(acts[:, :, n, :, 0:1], 0.0)                # left pad
                eng.memset(acts[:, :, n, :, w + 1 : WPAD], 0.0)       # right pad

            def emit_binarize(n, q):
                for a in range(CG):
                    xt = xs_pool.tile([P, hh, w], F32, tag="xstage")
                    nc.sync.dma_start(
                        out=xt[:],
                        in_=x_d[n, a * P : (a + 1) * P, q * hh : (q + 1) * hh, :],
                    )
                    nc.scalar.activation(
                        out=acts[
                            :, a, n, 1 + q * hh : 1 + (q + 1) * hh, 1 : w + 1
                        ],
                        in_=xt[:],
                        func=AF.Sign,
                    )

            # Critical path to the first matmul: image-0 margins, quarter-0
            # DMA+binarize of both groups, and the weights. Emit those first.
            emit_zero_margins(0, nc.vector)
            emit_binarize(0, 0)
            wsb = singles.tile([P, CG, 9, C], FP8)
            nc.sync.dma_start(out=wsb[:], in_=w_d)
            coef = singles.tile([P, CG, 3], F32)
            nc.sync.dma_start(out=coef[:], in_=coef_d)
            for n in range(1, b_per_core):
                emit_zero_margins(n, nc.gpsimd if n % 2 else nc.vector)
            for q in range(1, NQ):
                emit_binarize(0, q)
            for q in range(NQ):
                emit_binarize(1, q)

            # ---- conv + BN, pipelined per output-channel group ----
            # Group b=0's stats AllGather + normalize + DMA-out hide under
            # group b=1's conv; only group 1's tail is exposed.
            ybuf = singles.tile([P, CG, NT, FREE], F32)
            sum_p = small.tile([P, CG, NT], F32)
            sumsq_p = small.tile([P, CG, NT], F32)
            eps_t = small.tile([P, 1], F32)
            nc.vector.memset(eps_t[:], BN_EPS)

            flatacts = [
                acts[:, :, n, :, :].rearrange("p g h w -> p g (h w)")
                for n in range(b_per_core)
            ]

            for b in range(CG):
                for n in range(b_per_core):
                    if b == 0 and n + 2 < b_per_core:
                        for q in range(NQ):
                            emit_binarize(n + 2, q)
                    for t in range(tiles_per_img):
                        r0 = t * RT
                        # DoubleRow: contract both input channel groups at
                        # once. Moving AP must be flat 3D [K, 2, N]:
                        # contiguous 8x58 row-blocks (16 garbage cols,
                        # dropped at eviction).
                        ps = psum_pool.tile([P, FREE_PS], F32, tag="ps")
                        k = 0
                        for kh in range(3):
                            for kw in range(3):
                                st = (r0 + kh) * WPAD + kw
                                nc.tensor.matmul(
                                    ps[:],
                                    lhsT=wsb[
                                        :, :, kh * 3 + kw, b * P : (b + 1) * P
                                    ],
                                    rhs=flatacts[n][:, :, st : st + FREE_PS],
                                    start=(k == 0),
                                    stop=(k == 8),
                                    perf_mode=mybir.MatmulPerfMode.DoubleRow,
                                )
                                k += 1
                        idx = n * tiles_per_img + t
                        ps_v = ps[:].rearrange(
                            "p (r c) -> p r c", r=RT
                        )[:, :, 0:w]
                        # evict: copy PSUM->SBUF + per-channel sum (VectorE)
                        nc.vector.tensor_scalar(
                            out=ybuf[:, b, idx, :],
                            in0=ps_v,
                            scalar1=0.0,
                            scalar2=None,
                            op0=add,
                            op1=add,
                            accum_out=sum_p[:, b, idx : idx + 1],
                        )
                        # square + per-channel sumsq (ScalarE)
                        sqt = sq_pool.tile([P, FREE], F32, tag="sq")
                        nc.scalar.activation(
                            out=sqt[:],
                            in_=ps_v,
                            func=AF.Square,
                            accum_out=sumsq_p[:, b, idx : idx + 1],
                        )

                # ---- this group's stats: reduce, AllGather, local reduce ----
                stats_b = small.tile([P, 2], F32, tag=f"stats{b}")
                nc.vector.tensor_reduce(
                    out=stats_b[:, 0:1], in_=sum_p[:, b, :],
                    axis=mybir.AxisListType.X, op=add,
                )
                nc.vector.tensor_reduce(
                    out=stats_b[:, 1:2], in_=sumsq_p[:, b, :],
                    axis=mybir.AxisListType.X, op=add,
                )
                in_bounce = dram.tile([P, 2], F32, tag=f"inb{b}")
                out_bounce = dram.tile([n_cores * P, 2], F32, tag=f"outb{b}")
                nc.gpsimd.dma_start(out=in_bounce[:], in_=stats_b[:])
                nc.gpsimd.collective_compute(
                    "AllGather",
                    mybir.AluOpType.bypass,
                    replica_groups=[list(range(n_cores))],
                    ins=[in_bounce.opt()],
                    outs=[out_bounce.opt()],
                )
                gst8 = small.tile([P, 2, n_cores], F32, tag=f"gst8{b}")
                nc.gpsimd.dma_start(
                    out=gst8[:],
                    in_=out_bounce[:].rearrange("(c p) s -> p s c", c=n_cores),
                )
                gstats = small.tile([P, 2], F32, tag=f"gstats{b}")
                nc.vector.tensor_reduce(
                    out=gstats[:], in_=gst8[:], axis=mybir.AxisListType.X, op=add
                )

                # ---- per-channel affine coefficients for this group ----
                # mean = sum/nhw; ex2 = sumsq/nhw
                # var_y = (ex2 - mean^2)*sw^2 = (mean^2 - ex2)*negsw2
                # rstd = 1/sqrt(var_y+eps); a = ga*rstd (ga = gamma*sw)
                # bneg = mean*a - beta;  yn = y*a - bneg
                cf = small.tile([P, 6], F32, tag=f"cf{b}")
                mean_t, ex2_t, var_t, rstd_t, a_t, bneg_t = (
                    cf[:, i : i + 1] for i in range(6)
                )
                bpos_t = small.tile([P, 1], F32, tag=f"bp{b}")
                nc.vector.tensor_scalar_mul(
                    mean_t, gstats[:, 0:1], 1.0 / nhw_total
                )
                nc.vector.tensor_scalar_mul(
                    ex2_t, gstats[:, 1:2], 1.0 / nhw_total
                )
                nc.vector.scalar_tensor_tensor(
                    out=var_t, in0=mean_t, scalar=mean_t, in1=ex2_t,
                    op0=mult, op1=subtract,
                )
                nc.vector.tensor_tensor(
                    out=var_t, in0=var_t, in1=coef[:, b, 2:3], op=mult
                )
                nc.scalar.activation(
                    out=rstd_t, in_=var_t, func=AF.Sqrt, bias=eps_t[:], scale=1.0
                )
                nc.vector.reciprocal(out=rstd_t, in_=rstd_t)
                nc.vector.tensor_tensor(
                    out=a_t, in0=coef[:, b, 0:1], in1=rstd_t, op=mult
                )
                nc.vector.scalar_tensor_tensor(
                    out=bneg_t, in0=mean_t, scalar=a_t, in1=coef[:, b, 1:2],
                    op0=mult, op1=subtract,
                )
                nc.vector.tensor_scalar_mul(bpos_t[:], bneg_t, -1.0)

                # ---- apply affine + hardtanh, stream out ----
                # Per image: tiles [0:4) on DVE (both passes), tiles [4:7) as
                # Act affine -> Pool clip, so the exposed tail for group 1
                # runs on three engines concurrently.
                CH_A, CH_B = 4, tiles_per_img - 4
                for n in range(b_per_core):
                    idx = n * tiles_per_img
                    sta = stage_pool.tile([P, CH_A * FREE], F32, tag="affA")
                    nc.vector.tensor_scalar(
                        out=sta[:],
                        in0=ybuf[:, b, idx : idx + CH_A, :],
                        scalar1=a_t,
                        scalar2=bneg_t,
                        op0=mult,
                        op1=subtract,
                    )
                    sta2 = stage_pool.tile([P, CH_A * FREE], F32, tag="clipA")
                    nc.vector.tensor_scalar(
                        out=sta2[:],
                        in0=sta[:],
                        scalar1=1.0,
                        scalar2=-1.0,
                        op0=amin,
                        op1=amax,
                    )
                    nc.sync.dma_start(
                        out=out_d[
                            n, b * P : (b + 1) * P, 0 : CH_A * RT, :
                        ],
                        in_=sta2[:],
                    )
                    stb = stage_pool.tile([P, CH_B * FREE], F32, tag="affB")
                    nc.scalar.activation(
                        out=stb[:],
                        in_=ybuf[:, b, idx + CH_A : idx + tiles_per_img, :],
                        func=AF.Identity,
                        bias=bpos_t[:],
                        scale=a_t,
                    )
                    stb2 = stage_pool.tile([P, CH_B * FREE], F32, tag="clipB")
                    nc.gpsimd.tensor_scalar(
                        out=stb2[:],
                        in0=stb[:],
                        scalar1=1.0,
                        scalar2=-1.0,
                        op0=amin,
                        op1=amax,
                    )
                    nc.sync.dma_start(
                        out=out_d[
                            n, b * P : (b + 1) * P, CH_A * RT : h, :
                        ],
                        in_=stb2[:],
                    )

    nc.compile()
    return nc


def prep_inputs(x, weight, gamma, beta, b_per_core, n_cores):
    """Host-side prep: weight standardization/sign/scale + sharding."""
    w64 = np.asarray(weight, dtype=np.float64)
    co = w64.shape[0]
    wf = w64.reshape(co, -1)
    mean = wf.mean(axis=1)
    bw = w64 - mean[:, None, None, None]
    std = bw.reshape(co, -1).std(axis=1, ddof=1)
    mb = np.abs(bw / std[:, None, None, None]).reshape(co, -1).mean(axis=1)
    sw = 2.0 ** np.round(np.log2(mb))
    sgn = np.sign(bw)  # {-1, 0, +1}

    # wsgn[p, a, t, co] = sgn[co, a*128+p, kh, kw]
    s = sgn.reshape(co, CG, P, 9)
    wsgn = np.ascontiguousarray(s.transpose(2, 1, 3, 0)).astype(
        ml_dtypes.float8_e4m3
    )

    ga = (np.asarray(gamma, dtype=np.float64) * sw).astype(np.float32)
    be = np.asarray(beta, dtype=np.float32)
    negsw2 = (-sw * sw).astype(np.float32)
    coef = np.stack(
        [
            ga.reshape(CG, P).T,       # [p, g]  gamma*sw
            be.reshape(CG, P).T,       # beta
            negsw2.reshape(CG, P).T,   # -sw^2
        ],
        axis=-1,
    ).astype(np.float32)               # [P, CG, 3]

    x = np.asarray(x, dtype=np.float32)
    in_maps = []
    for c in range(n_cores):
        in_maps.append(
            {
                "x": np.ascontiguousarray(
                    x[c * b_per_core : (c + 1) * b_per_core]
                ),
                "wsgn": wsgn,
                "coef": coef,
            }
        )
    return in_maps


_CACHE = {}


def _get_nc(key, **kw):
    if key not in _CACHE:
        _CACHE[key] = build_kernel(**kw)
    return _CACHE[key]


def run(x, weight, gamma, beta, use_fp8=True, trace=False):
    assert use_fp8, "bf16 path removed"
    n, c, h, w = x.shape
    b_per_core = n // N_CORES
    nc = _get_nc(
        (b_per_core, h, w),
        b_per_core=b_per_core,
        h=h,
        w=w,
        n_cores=N_CORES,
    )
    in_maps = prep_inputs(x, weight, gamma, beta, b_per_core, N_CORES)
    res = run_bass_kernel_spmd(nc, in_maps, list(range(N_CORES)), trace=trace)
    out = np.concatenate([r["out"] for r in res.results], axis=0)
    return out, res


def kernel(x, weight, gamma, beta):
    out, _ = run(x, weight, gamma, beta)
    return out


# revision 11
# speedup vs baseline: 1.0452x; 1.0452x over previous
"""Trainium2 Bass kernel for IR-Net style binarized conv block.

Computation (matches the reference nn.Module):
  1. Per-out-channel weight standardization -> sign -> {-1,+1}, power-of-2
     per-channel scale sw (host-side numpy; weights are tiny).
  2. ba = sign(x) (device, ScalarE Sign activation, exact in fp8).
  3. y = conv2d(ba, sign_w) * sw  -- 3x3, pad 1, stride 1. Done as 9 shifted
     matmuls over a zero-padded SBUF activation image, channels on the
     partition axis, accumulating in PSUM. fp8 DoubleRow contracts both
     128-channel input groups at once. Exact: products are +-1 summed in
     fp32 PSUM.
  4. Training-mode BatchNorm over the FULL batch: per-channel sum / sumsq
     accumulate on-device, AllGather'd across the 8 cores (1KB), folded with
     sw, gamma, beta into per-channel affine a*z - bneg.
  5. Hardtanh clip via tensor_scalar(min,max).

Sharding: pure data parallel, batch 32 -> 4 images per core x 8 cores.

v2 over baseline:
  - WPAD=58 (=w+2, minimal padding; HP=64 keeps planes 16B-aligned) cuts the
    garbage columns in each matmul from 32/480 to 16/464.
  - Startup: image-0 quarter-0 DMAs + binarize are emitted first so the
    first matmul can start ~4us in instead of ~17us.
  - Stats bounce DMAs ride the Pool queue (Act was blocked on an
    ACT_TABLE_LOAD right when the tail collective had to launch).
  - Normalize (affine+clip) is split across DVE / Act / Pool per image so
    the tail isn't serialized on DVE.
  - Coefficient chain is fewer ops, mostly on DVE.
"""

import numpy as np
import ml_dtypes

import concourse.bacc as bacc
import concourse.bass as bass
import concourse.tile as tile
from concourse import mybir
from concourse.bass_utils import run_bass_kernel_spmd

F32 = mybir.dt.float32
FP8 = mybir.dt.float8e4

P = 128          # SBUF partitions
CG = 2           # channel groups: 256 channels = 2 x 128
C = 256
BN_EPS = 1e-5
N_CORES = 8
RT = 8           # output rows per PSUM tile


def build_kernel(b_per_core=4, h=56, w=56, n_cores=N_CORES):
    """Build the per-core Bass program. Returns the compiled Bacc instance."""
    WPAD = w + 2                        # 58: exactly left pad + data + right pad
    # fp8 DoubleRow moving AP is a flat contiguous block that can overrun the
    # last padded row by up to kh*WPAD+kw; give each image spare zero rows and
    # keep the per-image plane 16B-aligned (HP*WPAD % 16 == 0).
    HP = 64
    assert (HP * WPAD) % 16 == 0 and HP >= h + 4
    assert h % RT == 0
    tiles_per_img = h // RT
    NT = b_per_core * tiles_per_img     # PSUM tiles per output-channel group
    FREE_PS = RT * WPAD                 # moving free dim per matmul (464)
    FREE = RT * w                       # useful elems per tile (448)
    assert FREE_PS <= 512
    nhw_total = n_cores * b_per_core * h * w

    nc = bacc.Bacc(
        "TRN2", target_bir_lowering=False, debug=False, num_devices=n_cores
    )
    x_d = nc.dram_tensor("x", [b_per_core, C, h, w], F32, kind="ExternalInput").ap()
    w_d = nc.dram_tensor("wsgn", [P, CG, 9, C], FP8, kind="ExternalInput").ap()
    coef_d = nc.dram_tensor("coef", [P, CG, 3], F32, kind="ExternalInput").ap()
    out_d = nc.dram_tensor(
        "out", [b_per_core, C, h, w], F32, kind="ExternalOutput"
    ).ap()

    mult = mybir.AluOpType.mult
    add = mybir.AluOpType.add
    subtract = mybir.AluOpType.subtract
    amin = mybir.AluOpType.min
    amax = mybir.AluOpType.max
    AF = mybir.ActivationFunctionType
    NQ = 4
    hh = h // NQ                        # binarize chunk rows (14)

    with tile.TileContext(nc) as tc:
        with (
            tc.tile_pool(name="singles", bufs=1) as singles,
            tc.tile_pool(name="xs", bufs=4) as xs_pool,
            tc.tile_pool(name="psum", bufs=8, space="PSUM") as psum_pool,
            tc.tile_pool(name="sq", bufs=2) as sq_pool,
            tc.tile_pool(name="stage", bufs=2) as stage_pool,
            tc.tile_pool(name="small", bufs=1) as small,
            tc.tile_pool(name="dram", bufs=1, space="DRAM") as dram,
        ):
            # ---- padded, binarized activations (resident) ----
            acts = singles.tile([P, CG, b_per_core, HP, WPAD], FP8)

            def emit_zero_margins(n, eng):
                eng.memset(acts[:, :, n, 0, :], 0.0)                  # top pad
                eng.memset(acts[:, :, n, h + 1 : h + 5, :], 0.0)      # bottom pad + spill rows
                eng.memset(acts[:, :, n, :, 0:1], 0.0)                # left pad
                eng.memset(acts[:, :, n, :, w + 1 : WPAD], 0.0)       # right pad

            def emit_binarize(n, q):
                for a in range(CG):
                    xt = xs_pool.tile([P, hh, w], F32, tag="xstage")
                    # alternate input DMA between the SP and Pool queues so
                    # the 12.8MB stream isn't throttled by one DGE ring
                    eng = nc.sync if (n * NQ + q + a) % 2 == 0 else nc.gpsimd
                    eng.dma_start(
                        out=xt[:],
                        in_=x_d[n, a * P : (a + 1) * P, q * hh : (q + 1) * hh, :],
                    )
                    nc.scalar.activation(
                        out=acts[
                            :, a, n, 1 + q * hh : 1 + (q + 1) * hh, 1 : w + 1
                        ],
                        in_=xt[:],
                        func=AF.Sign,
                    )

            # Critical path to the first matmul: weights (Pool queue, parallel
            # with x on the SP queue), image-0 margins, quarter-0
            # DMA+binarize of both groups. Emit those first.
            wsb = singles.tile([P, CG, 9, C], FP8)
            nc.gpsimd.dma_start(out=wsb[:], in_=w_d)
            coef = singles.tile([P, CG, 3], F32)
            nc.gpsimd.dma_start(out=coef[:], in_=coef_d)
            emit_zero_margins(0, nc.vector)
            emit_binarize(0, 0)
            for n in range(1, b_per_core):
                emit_zero_margins(n, nc.gpsimd if n % 2 else nc.vector)
            for q in range(1, NQ):
                emit_binarize(0, q)
            for q in range(NQ):
                emit_binarize(1, q)

            # ---- conv + BN, pipelined per output-channel group ----
            # Group b=0's stats AllGather + normalize + DMA-out hide under
            # group b=1's conv; only group 1's tail is exposed.
            ybuf = singles.tile([P, CG, NT, FREE], F32)
            sum_p = small.tile([P, CG, NT], F32)
            sumsq_p = small.tile([P, CG, NT], F32)
            eps_t = small.tile([P, 1], F32)
            nc.vector.memset(eps_t[:], BN_EPS)

            flatacts = [
                acts[:, :, n, :, :].rearrange("p g h w -> p g (h w)")
                for n in range(b_per_core)
            ]

            for b in range(CG):
                for n in range(b_per_core):
                    if b == 0 and n + 2 < b_per_core:
                        for q in range(NQ):
                            emit_binarize(n + 2, q)
                    # k-outer over half-image tile blocks: load each of the 9
                    # stationary weight mats once per block and stream the
                    # block's row-tiles through it (amortizes LDWEIGHTS;
                    # back-to-back matmuls with the same weights run at full
                    # PE rate). DoubleRow: contract both input channel groups
                    # at once. Moving AP must be flat 3D [K, 2, N]:
                    # contiguous 8x58 row-blocks (16 garbage cols, dropped at
                    # eviction).
                    for t0, t1 in ((0, 4), (4, tiles_per_img)):
                        pss = [
                            psum_pool.tile([P, FREE_PS], F32, tag="ps", name="ps")
                            for _ in range(t1 - t0)
                        ]
                        for k in range(9):
                            kh, kw = divmod(k, 3)
                            for t in range(t0, t1):
                                st = (t * RT + kh) * WPAD + kw
                                nc.tensor.matmul(
                                    pss[t - t0][:],
                                    lhsT=wsb[:, :, k, b * P : (b + 1) * P],
                                    rhs=flatacts[n][:, :, st : st + FREE_PS],
                                    start=(k == 0),
                                    stop=(k == 8),
                                    perf_mode=mybir.MatmulPerfMode.DoubleRow,
                                )
                                if k == 8:
                                    # evict as soon as accumulation ends
                                    idx = n * tiles_per_img + t
                                    ps_v = pss[t - t0][:].rearrange(
                                        "p (r c) -> p r c", r=RT
                                    )[:, :, 0:w]
                                    # copy PSUM->SBUF + per-ch sum (VectorE)
                                    nc.vector.tensor_scalar(
                                        out=ybuf[:, b, idx, :],
                                        in0=ps_v,
                                        scalar1=0.0,
                                        scalar2=None,
                                        op0=add,
                                        op1=add,
                                        accum_out=sum_p[:, b, idx : idx + 1],
                                    )
                                    # square + per-ch sumsq (ScalarE)
                                    sqt = sq_pool.tile([P, FREE], F32, tag="sq")
                                    nc.scalar.activation(
                                        out=sqt[:],
                                        in_=ps_v,
                                        func=AF.Square,
                                        accum_out=sumsq_p[:, b, idx : idx + 1],
                                    )

                # ---- this group's stats: reduce, AllGather, local reduce ----
                stats_b = small.tile([P, 2], F32, tag=f"stats{b}")
                nc.vector.tensor_reduce(
                    out=stats_b[:, 0:1], in_=sum_p[:, b, :],
                    axis=mybir.AxisListType.X, op=add,
                )
                nc.vector.tensor_reduce(
                    out=stats_b[:, 1:2], in_=sumsq_p[:, b, :],
                    axis=mybir.AxisListType.X, op=add,
                )
                in_bounce = dram.tile([P, 2], F32, tag=f"inb{b}")
                out_bounce = dram.tile([n_cores * P, 2], F32, tag=f"outb{b}")
                nc.gpsimd.dma_start(out=in_bounce[:], in_=stats_b[:])
                nc.gpsimd.collective_compute(
                    "AllGather",
                    mybir.AluOpType.bypass,
                    replica_groups=[list(range(n_cores))],
                    ins=[in_bounce.opt()],
                    outs=[out_bounce.opt()],
                )
                gst8 = small.tile([P, 2, n_cores], F32, tag=f"gst8{b}")
                nc.gpsimd.dma_start(
                    out=gst8[:],
                    in_=out_bounce[:].rearrange("(c p) s -> p s c", c=n_cores),
                )
                gstats = small.tile([P, 2], F32, tag=f"gstats{b}")
                nc.vector.tensor_reduce(
                    out=gstats[:], in_=gst8[:], axis=mybir.AxisListType.X, op=add
                )

                # ---- per-channel affine coefficients for this group ----
                # mean = sum/nhw; ex2 = sumsq/nhw
                # var_y = (ex2 - mean^2)*sw^2 = (mean^2 - ex2)*negsw2
                # rstd = 1/sqrt(var_y+eps); a = ga*rstd (ga = gamma*sw)
                # bneg = mean*a - beta;  yn = y*a - bneg
                cf = small.tile([P, 6], F32, tag=f"cf{b}")
                mean_t, ex2_t, var_t, rstd_t, a_t, bneg_t = (
                    cf[:, i : i + 1] for i in range(6)
                )
                bpos_t = small.tile([P, 1], F32, tag=f"bp{b}")
                nc.vector.tensor_scalar_mul(
                    mean_t, gstats[:, 0:1], 1.0 / nhw_total
                )
                nc.vector.tensor_scalar_mul(
                    ex2_t, gstats[:, 1:2], 1.0 / nhw_total
                )
                nc.vector.scalar_tensor_tensor(
                    out=var_t, in0=mean_t, scalar=mean_t, in1=ex2_t,
                    op0=mult, op1=subtract,
                )
                nc.vector.tensor_tensor(
                    out=var_t, in0=var_t, in1=coef[:, b, 2:3], op=mult
                )
                nc.scalar.activation(
                    out=rstd_t, in_=var_t, func=AF.Sqrt, bias=eps_t[:], scale=1.0
                )
                nc.vector.reciprocal(out=rstd_t, in_=rstd_t)
                nc.vector.tensor_tensor(
                    out=a_t, in0=coef[:, b, 0:1], in1=rstd_t, op=mult
                )
                nc.vector.scalar_tensor_tensor(
                    out=bneg_t, in0=mean_t, scalar=a_t, in1=coef[:, b, 1:2],
                    op0=mult, op1=subtract,
                )
                nc.vector.tensor_scalar_mul(bpos_t[:], bneg_t, -1.0)

                # ---- apply affine + hardtanh, stream out ----
                # Per image: tiles [0:4) on DVE (both passes), tiles [4:7) as
                # Act affine -> Pool clip, so the exposed tail for group 1
                # runs on three engines concurrently.
                CH_A, CH_B = 4, tiles_per_img - 4
                # The tail (b=1) drains 6.4MB: put its B-chunk output DMAs on
                # the Act queue so two DGE rings share the writeback.
                out_q_b = nc.scalar if b == 1 else nc.sync
                for n in range(b_per_core):
                    idx = n * tiles_per_img
                    sta = stage_pool.tile([P, CH_A * FREE], F32, tag="affA")
                    nc.vector.tensor_scalar(
                        out=sta[:],
                        in0=ybuf[:, b, idx : idx + CH_A, :],
                        scalar1=a_t,
                        scalar2=bneg_t,
                        op0=mult,
                        op1=subtract,
                    )
                    nc.vector.tensor_scalar(
                        out=sta[:],
                        in0=sta[:],
                        scalar1=1.0,
                        scalar2=-1.0,
                        op0=amin,
                        op1=amax,
                    )
                    nc.sync.dma_start(
                        out=out_d[
                            n, b * P : (b + 1) * P, 0 : CH_A * RT, :
                        ],
                        in_=sta[:],
                    )
                    stb = stage_pool.tile([P, CH_B * FREE], F32, tag="affB")
                    nc.scalar.activation(
                        out=stb[:],
                        in_=ybuf[:, b, idx + CH_A : idx + tiles_per_img, :],
                        func=AF.Identity,
                        bias=bpos_t[:],
                        scale=a_t,
                    )
                    nc.gpsimd.tensor_scalar(
                        out=stb[:],
                        in0=stb[:],
                        scalar1=1.0,
                        scalar2=-1.0,
                        op0=amin,
                        op1=amax,
                    )
                    out_q_b.dma_start(
                        out=out_d[
                            n, b * P : (b + 1) * P, CH_A * RT : h, :
                        ],
                        in_=stb[:],
                    )

    nc.compile()
    return nc


def prep_inputs(x, weight, gamma, beta, b_per_core, n_cores):
    """Host-side prep: weight standardization/sign/scale + sharding."""
    w64 = np.asarray(weight, dtype=np.float64)
    co = w64.shape[0]
    wf = w64.reshape(co, -1)
    mean = wf.mean(axis=1)
    bw = w64 - mean[:, None, None, None]
    std = bw.reshape(co, -1).std(axis=1, ddof=1)
    mb = np.abs(bw / std[:, None, None, None]).reshape(co, -1).mean(axis=1)
    sw = 2.0 ** np.round(np.log2(mb))
    sgn = np.sign(bw)  # {-1, 0, +1}

    # wsgn[p, a, t, co] = sgn[co, a*128+p, kh, kw]
    s = sgn.reshape(co, CG, P, 9)
    wsgn = np.ascontiguousarray(s.transpose(2, 1, 3, 0)).astype(
        ml_dtypes.float8_e4m3
    )

    ga = (np.asarray(gamma, dtype=np.float64) * sw).astype(np.float32)
    be = np.asarray(beta, dtype=np.float32)
    negsw2 = (-sw * sw).astype(np.float32)
    coef = np.stack(
        [
            ga.reshape(CG, P).T,       # [p, g]  gamma*sw
            be.reshape(CG, P).T,       # beta
            negsw2.reshape(CG, P).T,   # -sw^2
        ],
        axis=-1,
    ).astype(np.float32)               # [P, CG, 3]

    x = np.asarray(x, dtype=np.float32)
    in_maps = []
    for c in range(n_cores):
        in_maps.append(
            {
                "x": np.ascontiguousarray(
                    x[c * b_per_core : (c + 1) * b_per_core]
                ),
                "wsgn": wsgn,
                "coef": coef,
            }
        )
    return in_maps


_CACHE = {}


def _get_nc(key, **kw):
    if key not in _CACHE:
        _CACHE[key] = build_kernel(**kw)
    return _CACHE[key]


def run(x, weight, gamma, beta, use_fp8=True, trace=False):
    assert use_fp8, "bf16 path removed"
    n, c, h, w = x.shape
    b_per_core = n // N_CORES
    nc = _get_nc(
        (b_per_core, h, w),
        b_per_core=b_per_core,
        h=h,
        w=w,
        n_cores=N_CORES,
    )
    in_maps = prep_inputs(x, weight, gamma, beta, b_per_core, N_CORES)
    res = run_bass_kernel_spmd(nc, in_maps, list(range(N_CORES)), trace=trace)
    out = np.concatenate([r["out"] for r in res.results], axis=0)
    return out, res


def kernel(x, weight, gamma, beta):
    out, _ = run(x, weight, gamma, beta)
    return out
